# revision 1
# baseline (speedup 1.0000x reference)
"""Distributed Bass kernel for nn_ANPM_81827716923970 (SimGNN-style graph-pair scorer).

8-way shard of the node dimension of both graphs. Per core:
  phase 1: DMA x shards (node-major bf16 + feat-major fp8) into SBUF,
           PE accumulates per-graph column sums (ones-weights matmuls).
  AG1:     AllGather colsum partials -> m1 -> h1 = tanh(m1 W0) (as 2*sig(2z)-1).
  pass 2:  PE s1 = X h1 (fp8 weights-form), ACT sigmoid -> a1,
           PE m2 += X^T a1 (a1-as-weights, bf16 rhs).
  AG2:     -> h2.
  pass 3:  s2raw = X h2; DVE s2 = a1*s2raw; ACT a2 = sigmoid; DVE w3 = a1*a2;
           PE out2 += X^T w3.
  AG3:     -> graph embeddings g1,g2 (ATT_NUM=2 duplication).
  NTN:     w_term (2 of 16 feature maps per core) + AG4 gather; v_term local;
           sigmoid MLP 16->8->4->2->1; DMA scalar out.
tanh via 2*sigmoid(2z)-1 => single ACT table set.
"""
import sys

import numpy as np

try:
    import concourse.bass as bass
except ImportError:
    sys.path.insert(0, "/opt/trn_rl_repo")
    import concourse.bass as bass

import ml_dtypes
import concourse.mybir as mybir

F32 = mybir.dt.float32
BF16 = mybir.dt.bfloat16
FP8 = mybir.dt.float8e4
AF = mybir.ActivationFunctionType
ALU = mybir.AluOpType

NC_CORES = 8
D = 256
T1, T2 = 120, 100
T = T1 + T2
N1, N2 = 120000, 100000
BS = 20
NB = T // BS          # 11
NB1 = T1 // BS        # 6
FDIM = T * D          # 56320

# smalls image columns
SM_W0C0, SM_W0C1, SM_ID = 0, 256, 512
SM_SEL0, SM_SEL1 = 640, 656
SM_V, SM_B = 672, 1696
SM_P0T, SM_P1T, SM_P2T, SM_P3T = 1697, 1705, 1709, 1711
SM = 1712

# ---- milestone values (must match emission order below) ----
# av (ACT):
AV_AG1, AV_MROW0, AV_MTC0, AV_HSIG0, AV_HCOLS0, AV_AG2 = 1, 2, 3, 4, 5, 6
AV_MROW1, AV_MTC1, AV_HSIG1, AV_HCOLS1, AV_AG3 = 7, 8, 9, 10, 11
AV_ROWS, AV_G1T, AV_X16, AV_MLP0, AV_MLP1, AV_MLP2, AV_FINAL = 12, 13, 14, 15, 16, 17, 18
AV_MROW = [AV_MROW0, AV_MROW1]
AV_MTC = [AV_MTC0, AV_MTC1]
AV_HSIG = [AV_HSIG0, AV_HSIG1]
AV_HCOLS = [AV_HCOLS0, AV_HCOLS1]
# pe (PE):
PE_COLSUM = 1
PE_SUM8 = [2, 7]
PE_MT = [3, 8]
PE_HMM = [4, 9]
PE_HT = [5, 10]
PE_M2, PE_O = 6, 11
PE_SUM8G, PE_G1T, PE_WTERM, PE_CB0, PE_CB1, PE_W16 = 12, 13, 14, 15, 16, 17
PE_MLP = [18, 19, 20, 21]
# vv (DVE):
VV_INIT = 1
VV_HROW = [2, 3]
def VV_S2M(b):  # pass-3 s2 mult for batch b complete
    return 4 + 2 * b
def VV_W3(b):
    return 5 + 2 * b
VV_AG4, VV_VMUL0, VV_SUM16 = 26, 27, 28
# dAG (sync collective-staging DMAs): in1=16,out1=32,in2=48,out2=64,
# in3=80,out3=96,in4=112,out4=128,out=144,dbg=160
DAG_AGOUT = [32, 64, 96, 128]


def build_nc():
    nc = bass.Bass()
    xnm = nc.declare_dram_parameter("xnm", [128, FDIM], BF16, isOutput=False)
    xfm = nc.declare_dram_parameter("xfm", [128, FDIM], FP8, isOutput=False)
    smalls = nc.declare_dram_parameter("smalls", [128, SM], F32, isOutput=False)
    wntn = nc.declare_dram_parameter("wntn", [128, 4096], F32, isOutput=False)
    out_ext = nc.declare_dram_parameter("out", [1, 1], F32, isOutput=True)
    dbg_ext = nc.declare_dram_parameter("dbg", [1, 2048], F32, isOutput=True)

    ag_in = [nc.dram_tensor(f"ag_in{k}", [1, 512], F32) for k in range(3)]
    ag_out = [nc.dram_tensor(f"ag_out{k}", [NC_CORES, 512], F32, addr_space="Shared")
              for k in range(3)]
    ag4_in = nc.dram_tensor("ag4_in", [1, 64], F32)
    ag4_out = nc.dram_tensor("ag4_out", [NC_CORES, 64], F32, addr_space="Shared")

    core_ids = list(range(NC_CORES))
    XH = FDIM // 4

    with (
        nc.Block() as block,
        nc.semaphore("dS") as dS,
        nc.semaphore("dA") as dA,
        nc.semaphore("dG") as dG,
        nc.semaphore("dAG") as dAG,
        nc.semaphore("cc") as cc,
        nc.semaphore("pe") as pe,
        nc.semaphore("av") as av,
        nc.semaphore("vv") as vv,
        nc.semaphore("sg") as sg,
        nc.semaphore("ps") as ps,
        nc.sbuf_tensor("xnmS", [128, FDIM], BF16) as xnmS,
        nc.sbuf_tensor("xfmS", [128, FDIM], FP8) as xfmS,
        nc.sbuf_tensor("smallsS", [128, SM], F32) as smallsS,
        nc.sbuf_tensor("wntnS", [128, 4096], F32) as wntnS,
        nc.sbuf_tensor("onescol_b", [128, 1], BF16) as onescol_b,
        nc.sbuf_tensor("onescol_f", [128, 1], F32) as onescol_f,
        nc.sbuf_tensor("onesrow_f", [1, 128], F32) as onesrow_f,
        nc.sbuf_tensor("idb", [128, 128], BF16) as idb,
        nc.sbuf_tensor("hcols", [128, 4], BF16) as hcols,
        nc.sbuf_tensor("abuf", [128, T], BF16) as abuf,
        nc.sbuf_tensor("a2buf", [128, T], BF16) as a2buf,
        nc.sbuf_tensor("w3buf", [128, T], BF16) as w3buf,
        nc.sbuf_tensor("s2m", [128, T], F32) as s2m,
        nc.sbuf_tensor("agst", [1, 512], F32) as agst,
        nc.sbuf_tensor("agbuf", [8, 512], F32) as agbuf,
        nc.sbuf_tensor("mrow", [1, 512], F32) as mrow,
        nc.sbuf_tensor("hsig", [1, 512], F32) as hsig,
        nc.sbuf_tensor("hrow", [1, 512], BF16) as hrow,
        nc.sbuf_tensor("mtc", [128, 4], F32) as mtc,
        nc.sbuf_tensor("grow", [1, 512], F32) as grow,
        nc.sbuf_tensor("g2row", [1, 512], F32) as g2row,
        nc.sbuf_tensor("crow", [1, 1024], F32) as crow,
        nc.sbuf_tensor("dotscr", [1, 512], F32) as dotscr,
        nc.sbuf_tensor("g1t", [128, 2], F32) as g1t,
        nc.sbuf_tensor("vprod", [16, 1024], F32) as vprod,
        nc.sbuf_tensor("vcol", [128, 1], F32) as vcol,
        nc.sbuf_tensor("sum16", [128, 1], F32) as sum16,
        nc.sbuf_tensor("x16", [128, 1], F32) as x16,
        nc.sbuf_tensor("mlpbuf", [128, 4], F32) as mlpbuf,
        nc.sbuf_tensor("ag4st", [1, 64], F32) as ag4st,
        nc.sbuf_tensor("w8buf", [8, 64], F32) as w8buf,
        nc.sbuf_tensor("outsb", [1, 1], F32) as outsb,
        nc.sbuf_tensor("dbgrow", [1, 2048], F32) as dbgrow,
        nc.psum_tensor("p_s0", [128, BS], F32) as p_s0,
        nc.psum_tensor("p_s1", [128, BS], F32) as p_s1,
        nc.psum_tensor("p_acc1", [1, 512], F32) as p_acc1,
        nc.psum_tensor("p_acc2", [1, 512], F32) as p_acc2,
        nc.psum_tensor("p_row", [1, 512], F32) as p_row,
        nc.psum_tensor("p_big", [128, 512], F32) as p_big,
        nc.psum_tensor("p_small", [128, 8], F32) as p_small,
    ):
        p_s = [p_s0, p_s1]
        W0c = [smallsS[:, SM_W0C0:SM_W0C0 + 256], smallsS[:, SM_W0C1:SM_W0C1 + 256]]
        id1f = smallsS[0:1, SM_ID:SM_ID + 1]

        # ---------------- SYNC ----------------
        @block.sync
        def _(sync):
            for b in range(0, NB, 2):
                sync.dma_start(out=xnmS[:, b * BS * D:(b + 1) * BS * D],
                               in_=xnm[:, b * BS * D:(b + 1) * BS * D]).then_inc(dS, 16)
            for j in (0, 2):
                sync.dma_start(out=xfmS[:, j * XH:(j + 1) * XH],
                               in_=xfm[:, j * XH:(j + 1) * XH]).then_inc(dS, 16)
            for k, (stv, ccv) in enumerate(((AV_AG1, 1), (AV_AG2, 2), (AV_AG3, 3))):
                sync.wait_ge(av, stv)
                sync.dma_start(out=ag_in[k][:], in_=agst.ap()).then_inc(dAG, 16)
                sync.wait_ge(cc, ccv)
                sync.dma_start(out=agbuf[:, :], in_=ag_out[k][:, :]).then_inc(dAG, 16)
            sync.wait_ge(vv, VV_AG4)
            sync.dma_start(out=ag4_in[:], in_=ag4st.ap()).then_inc(dAG, 16)
            sync.wait_ge(cc, 4)
            sync.dma_start(out=w8buf[:, :], in_=ag4_out[:, :]).then_inc(dAG, 16)
            sync.wait_ge(av, AV_FINAL)
            sync.dma_start(out=out_ext[:, :], in_=outsb.ap()).then_inc(dAG, 16)
            sync.dma_start(out=dbg_ext[:, :], in_=dbgrow.ap()).then_inc(dAG, 16)
            sync.wait_ge(dAG, 160)

        # ---------------- SCALAR (ACT + odd x DMAs) ----------------
        @block.scalar
        def _(scalar):
            for b in range(1, NB, 2):
                scalar.dma_start(out=xnmS[:, b * BS * D:(b + 1) * BS * D],
                                 in_=xnm[:, b * BS * D:(b + 1) * BS * D]).then_inc(dA, 16)
            for j in (1, 3):
                scalar.dma_start(out=xfmS[:, j * XH:(j + 1) * XH],
                                 in_=xfm[:, j * XH:(j + 1) * XH]).then_inc(dA, 16)

            scalar.wait_ge(pe, PE_COLSUM)
            scalar.activation(agst[0:1, 0:256], p_acc1[0:1, 0:256], AF.Copy)
            scalar.activation(agst[0:1, 256:512], p_acc2[0:1, 0:256], AF.Copy) \
                  .then_inc(av, 1)   # AV_AG1

            for k in (0, 1):
                scalar.wait_ge(pe, PE_SUM8[k])
                scalar.activation(dbgrow[0:1, k * 512:(k + 1) * 512], p_row[0:1, :],
                                  AF.Copy)
                scalar.activation(mrow[0:1, :], p_row[0:1, :], AF.Copy) \
                      .then_inc(av, 1)   # AV_MROW[k]
                scalar.wait_ge(pe, PE_MT[k])
                scalar.activation(mtc[:, 0:4], p_small[:, 0:4], AF.Copy) \
                      .then_inc(av, 1)   # AV_MTC[k]
                scalar.wait_ge(pe, PE_HMM[k])
                scalar.activation(hsig[0:1, 0:256], p_row[0:1, 0:256], AF.Sigmoid,
                                  scale=2.0 / N1)
                scalar.activation(hsig[0:1, 256:512], p_row[0:1, 256:512], AF.Sigmoid,
                                  scale=2.0 / N2).then_inc(av, 1)   # AV_HSIG[k]
                scalar.wait_ge(pe, PE_HT[k])
                scalar.activation(hcols[:, 0:4], p_small[:, 0:4], AF.Copy) \
                      .then_inc(av, 1)   # AV_HCOLS[k]
                for b in range(NB):
                    scalar.wait_ge(ps, k * NB + b + 1)
                    if k == 0:
                        scalar.activation(abuf[:, b * BS:(b + 1) * BS],
                                          p_s[b % 2][:, 0:BS], AF.Sigmoid) \
                              .then_inc(sg, 1)
                    else:
                        scalar.wait_ge(vv, VV_S2M(b))
                        scalar.activation(a2buf[:, b * BS:(b + 1) * BS],
                                          s2m[:, b * BS:(b + 1) * BS], AF.Sigmoid) \
                              .then_inc(sg, 1)
                if k == 0:
                    scalar.wait_ge(pe, PE_M2)
                    scalar.activation(agst[0:1, 0:256], p_acc1[0:1, 0:256], AF.Copy)
                    scalar.activation(agst[0:1, 256:512], p_acc2[0:1, 0:256], AF.Copy) \
                          .then_inc(av, 1)   # AV_AG2

            scalar.wait_ge(pe, PE_O)
            scalar.activation(agst[0:1, 0:256], p_acc1[0:1, 0:256], AF.Copy)
            scalar.activation(agst[0:1, 256:512], p_acc2[0:1, 0:256], AF.Copy) \
                  .then_inc(av, 1)   # AV_AG3

            scalar.wait_ge(pe, PE_SUM8G)
            scalar.activation(dbgrow[0:1, 1024:1536], p_row[0:1, :], AF.Copy)
            scalar.activation(grow[0:1, :], p_row[0:1, :], AF.Copy)
            scalar.activation(g2row[0:1, 0:256], grow[0:1, 256:512], AF.Copy)
            scalar.activation(g2row[0:1, 256:512], grow[0:1, 256:512], AF.Copy)
            scalar.activation(crow[0:1, 0:256], grow[0:1, 0:256], AF.Copy)
            scalar.activation(crow[0:1, 256:512], grow[0:1, 0:256], AF.Copy)
            scalar.activation(crow[0:1, 512:768], grow[0:1, 256:512], AF.Copy)
            scalar.activation(crow[0:1, 768:1024], grow[0:1, 256:512], AF.Copy) \
                  .then_inc(av, 1)   # AV_ROWS
            scalar.wait_ge(pe, PE_G1T)
            scalar.activation(g1t[:, 0:2], p_small[:, 0:2], AF.Copy) \
                  .then_inc(av, 1)   # AV_G1T

            scalar.wait_ge(vv, VV_SUM16)
            scalar.activation(x16[0:16, 0:1], sum16[0:16, 0:1], AF.Sigmoid) \
                  .then_inc(av, 1)   # AV_X16
            for li, width in enumerate((8, 4, 2, 1)):
                scalar.wait_ge(pe, PE_MLP[li])
                if li < 3:
                    scalar.activation(mlpbuf[0:width, li:li + 1],
                                      p_small[0:width, 4 + li:5 + li], AF.Sigmoid) \
                          .then_inc(av, 1)
                else:
                    scalar.activation(outsb[0:1, 0:1], p_small[0:1, 4 + li:5 + li],
                                      AF.Sigmoid).then_inc(av, 1)   # AV_FINAL

        # ---------------- GPSIMD ----------------
        @block.gpsimd
        def _(gpsimd):
            gpsimd.dma_start(out=smallsS[:, :], in_=smalls[:, :]).then_inc(dG, 16)
            gpsimd.wait_ge(dAG, 16)
            gpsimd.collective_compute(
                "AllGather", ALU.bypass, replica_groups=[core_ids],
                ins=[ag_in[0].ap().opt()], outs=[ag_out[0].ap().opt()]).then_inc(cc, 1)
            gpsimd.dma_start(out=wntnS[:, :], in_=wntn[:, :]).then_inc(dG, 16)
            gpsimd.wait_ge(dAG, 48)
            gpsimd.collective_compute(
                "AllGather", ALU.bypass, replica_groups=[core_ids],
                ins=[ag_in[1].ap().opt()], outs=[ag_out[1].ap().opt()]).then_inc(cc, 1)
            gpsimd.wait_ge(dAG, 80)
            gpsimd.collective_compute(
                "AllGather", ALU.bypass, replica_groups=[core_ids],
                ins=[ag_in[2].ap().opt()], outs=[ag_out[2].ap().opt()]).then_inc(cc, 1)
            gpsimd.wait_ge(dAG, 112)
            gpsimd.collective_compute(
                "AllGather", ALU.bypass, replica_groups=[core_ids],
                ins=[ag4_in.ap().opt()], outs=[ag4_out.ap().opt()]).then_inc(cc, 1)

        # ---------------- VECTOR ----------------
        @block.vector
        def _(vector):
            vector.memset(onescol_b[:, :], 1.0)
            vector.memset(onescol_f[:, :], 1.0)
            vector.memset(onesrow_f[:, :], 1.0)
            vector.memset(ag4st[:, :], 0.0)
            vector.wait_ge(dG, 16)
            vector.tensor_copy(idb[:, :], smallsS[:, SM_ID:SM_ID + 128]) \
                  .then_inc(vv, 1)   # VV_INIT
            for k in (0, 1):
                vector.wait_ge(av, AV_HSIG[k])
                vector.tensor_scalar(hrow[0:1, :], hsig[0:1, :], 2.0, -1.0,
                                     ALU.mult, ALU.add).then_inc(vv, 1)  # VV_HROW[k]
            for b in range(NB):
                vector.wait_ge(ps, NB + b + 1)
                vector.tensor_tensor(s2m[:, b * BS:(b + 1) * BS],
                                     p_s[b % 2][:, 0:BS],
                                     abuf[:, b * BS:(b + 1) * BS],
                                     op=ALU.mult).then_inc(vv, 1)   # VV_S2M(b)
                vector.wait_ge(sg, NB + b + 1)
                vector.tensor_tensor(w3buf[:, b * BS:(b + 1) * BS],
                                     abuf[:, b * BS:(b + 1) * BS],
                                     a2buf[:, b * BS:(b + 1) * BS],
                                     op=ALU.mult).then_inc(vv, 1)   # VV_W3(b)
            # w_term dot products -> ag4 payload (critical path first)
            vector.wait_ge(pe, PE_WTERM)
            vector.tensor_tensor(dotscr[0:1, 0:512], p_acc1[0:1, 0:512],
                                 g2row[0:1, :], op=ALU.mult)
            vector.tensor_reduce(ag4st[0:1, 0:1], dotscr[0:1, 0:512],
                                 axis=mybir.AxisListType.X, op=ALU.add)
            vector.tensor_tensor(dotscr[0:1, 0:512], p_acc2[0:1, 0:512],
                                 g2row[0:1, :], op=ALU.mult)
            vector.tensor_reduce(ag4st[0:1, 1:2], dotscr[0:1, 0:512],
                                 axis=mybir.AxisListType.X, op=ALU.add) \
                  .then_inc(vv, 1)   # VV_AG4
            # v_term
            vector.wait_ge(pe, PE_CB0)
            vector.tensor_tensor(vprod[0:16, 0:512], smallsS[0:16, SM_V:SM_V + 512],
                                 p_big[0:16, 0:512], op=ALU.mult) \
                  .then_inc(vv, 1)   # VV_VMUL0
            vector.wait_ge(pe, PE_CB1)
            vector.tensor_tensor(vprod[0:16, 512:1024],
                                 smallsS[0:16, SM_V + 512:SM_V + 1024],
                                 p_big[0:16, 0:512], op=ALU.mult)
            vector.tensor_reduce(vcol[0:16, 0:1], vprod[0:16, 0:1024],
                                 axis=mybir.AxisListType.X, op=ALU.add)
            vector.wait_ge(pe, PE_W16)
            vector.tensor_tensor(sum16[0:16, 0:1], vcol[0:16, 0:1],
                                 p_small[0:16, 0:1], op=ALU.add)
            vector.tensor_tensor(sum16[0:16, 0:1], sum16[0:16, 0:1],
                                 smallsS[0:16, SM_B:SM_B + 1], op=ALU.add) \
                  .then_inc(vv, 1)   # VV_SUM16

        # ---------------- TENSOR ----------------
        @block.tensor
        def _(tensor):
            def xta_batch(b, wbuf):
                acc = p_acc1 if b < NB1 else p_acc2
                mm = None
                for j in range(BS):
                    t = b * BS + j
                    tt = t if b < NB1 else t - T1
                    mm = tensor.matmul(acc[0:1, 0:256], wbuf[:, t:t + 1],
                                       xnmS[:, t * D:(t + 1) * D],
                                       start=(tt == 0),
                                       stop=(tt == (T1 if b < NB1 else T2) - 1))
                return mm

            tensor.wait_ge(vv, VV_INIT)
            for b in range(NB):
                if b % 2 == 0:
                    tensor.wait_ge(dS, 16 * (b // 2 + 1))
                else:
                    tensor.wait_ge(dA, 16 * ((b + 1) // 2))
                acc = p_acc1 if b < NB1 else p_acc2
                for j in range(BS):
                    t = b * BS + j
                    tt = t if b < NB1 else t - T1
                    mm = tensor.matmul(acc[0:1, 0:256], onescol_b[:, 0:1],
                                       xnmS[:, t * D:(t + 1) * D],
                                       start=(tt == 0),
                                       stop=(tt == (T1 if b < NB1 else T2) - 1))
            mm.then_inc(pe, 1)   # PE_COLSUM

            def h_chain(k):
                tensor.wait_ge(dAG, DAG_AGOUT[k])
                tensor.matmul(p_row[0:1, :], onescol_f[0:8, 0:1], agbuf[0:8, :],
                              start=True, stop=True).then_inc(pe, 1)   # PE_SUM8[k]
                tensor.wait_ge(av, AV_MROW[k])
                for j in range(4):
                    mm = tensor.matmul(p_small[:, j:j + 1],
                                       mrow[0:1, j * 128:(j + 1) * 128], id1f,
                                       start=(j == 0), stop=(j == 3),
                                       is_transpose=True)
                mm.then_inc(pe, 1)   # PE_MT[k]
                tensor.wait_ge(av, AV_MTC[k])
                for g in (0, 1):
                    for c in (0, 1):
                        mm = tensor.matmul(p_row[0:1, g * 256:(g + 1) * 256],
                                           mtc[:, 2 * g + c:2 * g + c + 1], W0c[c],
                                           start=(g == 0 and c == 0),
                                           stop=(g == 1 and c == 1))
                mm.then_inc(pe, 1)   # PE_HMM[k]
                tensor.wait_ge(vv, VV_HROW[k])
                for j in range(4):
                    mm = tensor.matmul(p_small[:, j:j + 1],
                                       hrow[0:1, j * 128:(j + 1) * 128], idb[0:1, 0:1],
                                       start=(j == 0), stop=(j == 3),
                                       is_transpose=True)
                mm.then_inc(pe, 1)   # PE_HT[k]

            def run_pass(k):
                wbuf = abuf if k == 0 else w3buf
                for b in range(NB):
                    g = 0 if b < NB1 else 1
                    if b == 0:
                        tensor.wait_ge(av, AV_HCOLS[k])
                        if k == 0:
                            tensor.wait_ge(dS, 16 * 8)
                            tensor.wait_ge(dA, 16 * 7)
                    if b >= 2:
                        if k == 0:
                            tensor.wait_ge(sg, k * NB + b - 1)
                        else:
                            tensor.wait_ge(vv, VV_S2M(b - 2))
                    mm = None
                    for c in (0, 1):
                        for j in range(BS):
                            t = b * BS + j
                            mm = tensor.matmul(
                                p_s[b % 2][:, j:j + 1],
                                xfmS[:, (2 * t + c) * 128:(2 * t + c + 1) * 128],
                                hcols[:, 2 * g + c:2 * g + c + 1],
                                start=(c == 0 and j == 0),
                                stop=(c == 1 and j == BS - 1))
                    mm.then_inc(ps, 1)
                    if b >= 1:
                        bb = b - 1
                        if k == 0:
                            tensor.wait_ge(sg, k * NB + bb + 1)
                        else:
                            tensor.wait_ge(vv, VV_W3(bb))
                        xta_batch(bb, wbuf)
                bb = NB - 1
                if k == 0:
                    tensor.wait_ge(sg, k * NB + bb + 1)
                else:
                    tensor.wait_ge(vv, VV_W3(bb))
                mm = xta_batch(bb, wbuf)
                mm.then_inc(pe, 1)   # PE_M2 / PE_O

            h_chain(0)
            run_pass(0)
            h_chain(1)
            run_pass(1)

            # ---- NTN ----
            tensor.wait_ge(dAG, DAG_AGOUT[2])
            tensor.matmul(p_row[0:1, :], onescol_f[0:8, 0:1], agbuf[0:8, :],
                          start=True, stop=True).then_inc(pe, 1)   # PE_SUM8G
            tensor.wait_ge(av, AV_ROWS)
            for j in range(2):
                mm = tensor.matmul(p_small[:, j:j + 1],
                                   grow[0:1, j * 128:(j + 1) * 128], id1f,
                                   start=(j == 0), stop=(j == 1), is_transpose=True)
            mm.then_inc(pe, 1)   # PE_G1T
            tensor.wait_ge(av, AV_G1T)
            tensor.wait_ge(dG, 32)
            for p in (0, 1):
                acc = p_acc1 if p == 0 else p_acc2
                for c in range(4):
                    mm = tensor.matmul(acc[0:1, 0:512], g1t[:, c % 2:c % 2 + 1],
                                       wntnS[:, (4 * p + c) * 512:(4 * p + c + 1) * 512],
                                       start=(c == 0), stop=(c == 3))
            mm.then_inc(pe, 1)   # PE_WTERM
            tensor.matmul(p_big[0:16, 0:512], onesrow_f[0:1, 0:16], crow[0:1, 0:512],
                          start=True, stop=True).then_inc(pe, 1)   # PE_CB0
            tensor.wait_ge(vv, VV_VMUL0)
            tensor.matmul(p_big[0:16, 0:512], onesrow_f[0:1, 0:16],
                          crow[0:1, 512:1024], start=True, stop=True) \
                  .then_inc(pe, 1)   # PE_CB1
            tensor.wait_ge(dAG, 128)
            for p in (0, 1):
                mm = tensor.matmul(p_small[0:16, 0:1],
                                   smallsS[0:8, SM_SEL0 + 16 * p:SM_SEL0 + 16 * (p + 1)],
                                   w8buf[0:8, p:p + 1], start=(p == 0), stop=(p == 1))
            mm.then_inc(pe, 1)   # PE_W16
            tensor.wait_ge(av, AV_X16)
            tensor.matmul(p_small[0:8, 4:5], smallsS[0:16, SM_P0T:SM_P0T + 8],
                          x16[0:16, 0:1], start=True, stop=True).then_inc(pe, 1)
            tensor.wait_ge(av, AV_MLP0)
            tensor.matmul(p_small[0:4, 5:6], smallsS[0:8, SM_P1T:SM_P1T + 4],
                          mlpbuf[0:8, 0:1], start=True, stop=True).then_inc(pe, 1)
            tensor.wait_ge(av, AV_MLP1)
            tensor.matmul(p_small[0:2, 6:7], smallsS[0:4, SM_P2T:SM_P2T + 2],
                          mlpbuf[0:4, 1:2], start=True, stop=True).then_inc(pe, 1)
            tensor.wait_ge(av, AV_MLP2)
            tensor.matmul(p_small[0:1, 7:8], smallsS[0:2, SM_P3T:SM_P3T + 1],
                          mlpbuf[0:2, 2:3], start=True, stop=True).then_inc(pe, 1)

    return nc


# ---------------- host-side prep ----------------
def _pad_rows(x, rows):
    out = np.zeros((rows, x.shape[1]), dtype=np.float32)
    out[: x.shape[0]] = x
    return out


def prep_inputs(x1, x2, W0, V, W, b, P0, P1, P2, P3):
    x1 = np.asarray(x1, np.float32)
    x2 = np.asarray(x2, np.float32)
    x1p = _pad_rows(x1, NC_CORES * T1 * 128)
    x2p = _pad_rows(x2, NC_CORES * T2 * 128)

    smalls = np.zeros((128, SM), np.float32)
    W0 = np.asarray(W0, np.float32)
    smalls[:, SM_W0C0:SM_W0C0 + 256] = W0[0:128, :]
    smalls[:, SM_W0C1:SM_W0C1 + 256] = W0[128:256, :]
    smalls[:, SM_ID:SM_ID + 128] = np.eye(128, dtype=np.float32)
    V = np.asarray(V, np.float32)
    smalls[0:16, SM_V:SM_V + 1024] = V
    smalls[0:16, SM_B] = np.asarray(b, np.float32)
    smalls[0:16, SM_P0T:SM_P0T + 8] = np.asarray(P0, np.float32).T
    smalls[0:8, SM_P1T:SM_P1T + 4] = np.asarray(P1, np.float32).T
    smalls[0:4, SM_P2T:SM_P2T + 2] = np.asarray(P2, np.float32).T
    smalls[0:2, SM_P3T:SM_P3T + 1] = np.asarray(P3, np.float32).T
    for p in (0, 1):
        sel = np.zeros((8, 16), np.float32)
        for c in range(8):
            sel[c, 2 * c + p] = 1.0
        smalls[0:8, SM_SEL0 + 16 * p:SM_SEL0 + 16 * (p + 1)] = sel

    W = np.asarray(W, np.float32)
    in_maps = []
    for c in range(NC_CORES):
        t1 = x1p[c * T1 * 128:(c + 1) * T1 * 128].reshape(T1, 128, D)
        t2 = x2p[c * T2 * 128:(c + 1) * T2 * 128].reshape(T2, 128, D)
        tiles = np.concatenate([t1, t2], axis=0)          # (T,128,256)
        xnm_c = tiles.transpose(1, 0, 2).reshape(128, FDIM)
        xfm_c = tiles.reshape(T, 128, 2, 128).transpose(3, 0, 2, 1).reshape(128, FDIM)
        wn = np.zeros((128, 4096), np.float32)
        for p in (0, 1):
            wf = W[2 * c + p]                             # (512,512)
            for ch in range(4):
                wn[:, (4 * p + ch) * 512:(4 * p + ch + 1) * 512] = \
                    wf[128 * ch:128 * (ch + 1), :]
        in_maps.append({
            "xnm": xnm_c.astype(ml_dtypes.bfloat16),
            "xfm": xfm_c.astype(ml_dtypes.float8_e4m3fn),
            "smalls": smalls,
            "wntn": wn,
        })
    return in_maps


_CACHE = {}


def kernel(x1, x2, W0, V, W, b, P0, P1, P2, P3, trace=False, tmpdir=None):
    from concourse.bass_utils import run_bass_kernel_spmd
    if "nc" not in _CACHE:
        _CACHE["nc"] = build_nc()
    nc = _CACHE["nc"]
    in_maps = prep_inputs(x1, x2, W0, V, W, b, P0, P1, P2, P3)
    res = run_bass_kernel_spmd(nc, in_maps, list(range(NC_CORES)),
                               trace=trace, tmpdir=tmpdir)
    kernel.last_result = res
    return np.asarray(res.results[0]["out"], np.float32)
